# revision 2
# baseline (speedup 1.0000x reference)
"""Distributed Trainium2 (8 NeuronCores) attention-head kernel.

Problem: single attention head with projections.
  q = Q @ Wq.T + bq ; k = K @ Wk.T + bk ; v = V @ Wv.T + bv
  x = (q @ k.T) / sqrt(64) ; x = x*m - 1e9*(1-m) ; p = softmax(x)
  y = p @ v
Shapes: Q/K/V [2, 4096, 1024] f32, mask [2, 4096, 4096] int32 -> y [2, 4096, 64] f32.

Strategy (8 cores): shard queries 8-ways (2 batches x 4 query-chunks of 1024
rows).  K/V are replicated within each 4-core batch group (collective_compute
has ~100us fixed overhead on this fleet; replication costs ~45us of extra DMA
at bf16 - cheaper and with no cross-core sync).  The host reshards into
matmul-native transposed layouts (contraction dim on partitions) so the device
does zero large transposes, and casts Q/K/V/W to bf16 and the 0/1 mask to
fp8e4 (exact) - the softmax algebra p=exp(x)*m, y=(p@v)/sum(p) is exactly
equivalent to the reference's masked softmax for non-degenerate rows.

Per-core pipeline (all layouts [partitions, free]):
  qT[64,1024] = sum_j WqT[j].T @ QT[j]      (dm-chunk j, PSUM accumulate)
  kT[64,4096], vT[64,4096] likewise; v_aug[s,65] = [v | 1] via PE transposes
  per s-chunk (32 x 128): sT = kT_chunk.T @ qT ; p = exp(sT/8) * maskT
                          yT[65,1024] += v_aug_chunk.T @ p   (PSUM accumulate)
  y[q,65] = transpose(yT); out = y[:, :64] / y[:, 64:65]
"""

import numpy as np
import ml_dtypes

import concourse.bass as bass
import concourse.mybir as mybir
import concourse.tile as tile
from concourse import bacc
from concourse.bass_utils import run_bass_kernel_spmd
from concourse.masks import make_identity

B, S, DM, DK = 2, 4096, 1024, 64
N_CORES = 8
GROUP = 4            # cores per batch
SQ = S // GROUP      # query rows per core (1024)
NDM = DM // 128      # dm chunks (8)
NSC = S // 128       # s chunks (32)

F32 = mybir.dt.float32
BF16 = mybir.dt.bfloat16
FP8 = mybir.dt.float8e4

_last_results = None


def _build():
    nc = bacc.Bacc(None, target_bir_lowering=False)

    qt_e = nc.declare_dram_parameter("qt", [DM, SQ], BF16, isOutput=False)
    kt_e = nc.declare_dram_parameter("kt", [DM, S], BF16, isOutput=False)
    vt_e = nc.declare_dram_parameter("vt", [DM, S], BF16, isOutput=False)
    mt_e = nc.declare_dram_parameter("mt", [S, SQ], FP8, isOutput=False)
    wq_e = nc.declare_dram_parameter("wq", [DM, DK], BF16, isOutput=False)
    wk_e = nc.declare_dram_parameter("wk", [DM, DK], BF16, isOutput=False)
    wv_e = nc.declare_dram_parameter("wv", [DM, DK], BF16, isOutput=False)
    bq_e = nc.declare_dram_parameter("bq", [DK, 1], F32, isOutput=False)
    bk_e = nc.declare_dram_parameter("bk", [DK, 1], F32, isOutput=False)
    bv_e = nc.declare_dram_parameter("bv", [DK, 1], F32, isOutput=False)
    out_e = nc.declare_dram_parameter("out", [SQ, DK], F32, isOutput=True)

    with tile.TileContext(nc) as tc:
        with (
            tc.tile_pool(name="const", bufs=1) as cpool,
            tc.tile_pool(name="mask", bufs=NSC) as mpool,
            tc.tile_pool(name="qin", bufs=3) as qpool,
            tc.tile_pool(name="kin", bufs=3) as kpool,
            tc.tile_pool(name="big", bufs=1) as bigpool,
            tc.tile_pool(name="pp", bufs=6) as ppool,
            tc.tile_pool(name="psum", bufs=1, space="PSUM") as pproj,
            tc.tile_pool(name="psum_s", bufs=4, space="PSUM") as psT,
            tc.tile_pool(name="psum_y", bufs=1, space="PSUM") as pyT,
        ):
            # ---- constants ----
            # weights reshaped [128, NDM*DK]: chunk j at [:, j*DK:(j+1)*DK]
            w_sb = {}
            for name, ext in (("wq", wq_e), ("wk", wk_e), ("wv", wv_e)):
                t = cpool.tile([128, NDM * DK], BF16, tag=f"w_{name}")
                nc.sync.dma_start(
                    t[:].rearrange("p (j f) -> p j f", j=NDM),
                    ext.rearrange("(j p) f -> p j f", p=128),
                )
                w_sb[name] = t
            b_sb = {}
            for name, ext in (("bq", bq_e), ("bk", bk_e), ("bv", bv_e)):
                t = cpool.tile([DK, 1], F32, tag=f"b_{name}")
                nc.sync.dma_start(t[:], ext[:])
                b_sb[name] = t
            ident_bf = cpool.tile([128, 128], BF16, tag="ident_bf")
            make_identity(nc, ident_bf[:])
            ident_f32 = cpool.tile([128, 128], F32, tag="ident_f32")
            make_identity(nc, ident_f32[:])

            # ---- mask loads (issued early; big pool keeps the whole mask) ----
            mt_tiles = []
            for j in range(NSC):
                mt = mpool.tile([128, SQ], FP8, tag="mt")
                nc.sync.dma_start(mt[:], mt_e[j * 128:(j + 1) * 128, :])
                mt_tiles.append(mt)

            # ---- projections ----
            def project(ext, wname, bname, dst_sb, s_cols):
                """dst_sb[64, s_cols] (bf16) = W @ X^T + b, X^T given as ext [DM, s_cols]."""
                ngrp = s_cols // 1024
                for g in range(ngrp):
                    ps = pproj.tile([DK, 1024], F32, tag="proj")
                    for j in range(NDM):
                        xin = (qpool if s_cols == SQ else kpool).tile(
                            [128, 1024], BF16, tag="xin"
                        )
                        nc.sync.dma_start(
                            xin[:],
                            ext[j * 128:(j + 1) * 128,
                                g * 1024:(g + 1) * 1024],
                        )
                        for h in range(2):
                            nc.tensor.matmul(
                                ps[:, h * 512:(h + 1) * 512],
                                lhsT=w_sb[wname][:, j * DK:(j + 1) * DK],
                                rhs=xin[:, h * 512:(h + 1) * 512],
                                start=(j == 0),
                                stop=(j == NDM - 1),
                            )
                    nc.vector.tensor_scalar_add(
                        dst_sb[:, g * 1024:(g + 1) * 1024], ps[:], b_sb[bname][:]
                    )

            qT_sb = bigpool.tile([DK, SQ], BF16, tag="qT")
            project(qt_e, "wq", "bq", qT_sb, SQ)
            kT_sb = bigpool.tile([DK, S], BF16, tag="kT")
            project(kt_e, "wk", "bk", kT_sb, S)
            vT_sb = bigpool.tile([DK, S], BF16, tag="vT")
            project(vt_e, "wv", "bv", vT_sb, S)

            # ---- v_aug[s, 65] tiles: transpose vT per s-chunk; col 64 = 1 ----
            v_aug = bigpool.tile([128, NSC * 65], BF16, tag="vaug")
            nc.vector.memset(v_aug[:], 1.0)
            for j in range(NSC):
                pt = psT.tile([128, DK], BF16, tag="sT")
                nc.tensor.transpose(
                    pt[:], vT_sb[:, j * 128:(j + 1) * 128], ident_bf[:DK, :DK]
                )
                nc.vector.tensor_copy(v_aug[:, j * 65:j * 65 + DK], pt[:])

            # ---- main loop: scores -> exp -> mask -> AV accumulate ----
            yT_ps = pyT.tile([65, SQ], F32, tag="yT")
            for j in range(NSC):
                for h in range(2):
                    sT = psT.tile([128, 512], F32, tag="sT")
                    nc.tensor.matmul(
                        sT[:],
                        lhsT=kT_sb[:, j * 128:(j + 1) * 128],
                        rhs=qT_sb[:, h * 512:(h + 1) * 512],
                        start=True,
                        stop=True,
                    )
                    p = ppool.tile([128, 512], BF16, tag="p")
                    nc.scalar.activation(
                        p[:], sT[:], mybir.ActivationFunctionType.Exp,
                        scale=0.125,
                    )
                    nc.vector.tensor_mul(
                        p[:], p[:], mt_tiles[j][:, h * 512:(h + 1) * 512]
                    )
                    nc.tensor.matmul(
                        yT_ps[:, h * 512:(h + 1) * 512],
                        lhsT=v_aug[:, j * 65:(j + 1) * 65],
                        rhs=p[:],
                        start=(j == 0),
                        stop=(j == NSC - 1),
                    )

            # ---- epilogue: y = transpose(yT); out = y[:, :64] / y[:, 64] ----
            yT_sb = bigpool.tile([65, SQ], F32, tag="yT_sb")
            nc.scalar.copy(yT_sb[:], yT_ps[:])
            for t in range(SQ // 128):
                yp = psT.tile([128, 65], F32, tag="sT")
                nc.tensor.transpose(
                    yp[:], yT_sb[:, t * 128:(t + 1) * 128],
                    ident_f32[:65, :65],
                )
                rcp = ppool.tile([128, 1], F32, tag="rcp")
                nc.vector.reciprocal(rcp[:], yp[:, DK:DK + 1])
                y_sb = ppool.tile([128, DK], F32, tag="y")
                nc.vector.tensor_scalar_mul(y_sb[:], yp[:, :DK], rcp[:])
                nc.sync.dma_start(out_e[t * 128:(t + 1) * 128, :], y_sb[:])

    nc.finalize()
    return nc


def kernel(Q, K, V, mask, Wq, bq, Wk, bk, Wv, bv):
    global _last_results
    bf16 = ml_dtypes.bfloat16
    fp8 = ml_dtypes.float8_e4m3

    wq_t = np.ascontiguousarray(Wq.T.astype(bf16))
    wk_t = np.ascontiguousarray(Wk.T.astype(bf16))
    wv_t = np.ascontiguousarray(Wv.T.astype(bf16))
    b_col = lambda b: np.ascontiguousarray(b.astype(np.float32).reshape(DK, 1))
    bqc, bkc, bvc = b_col(bq), b_col(bk), b_col(bv)

    kt_b = [np.ascontiguousarray(K[b].T.astype(bf16)) for b in range(B)]
    vt_b = [np.ascontiguousarray(V[b].T.astype(bf16)) for b in range(B)]

    in_maps = []
    for c in range(N_CORES):
        b, i = divmod(c, GROUP)
        rows = slice(i * SQ, (i + 1) * SQ)
        in_maps.append({
            "qt": np.ascontiguousarray(Q[b, rows, :].T.astype(bf16)),
            "kt": kt_b[b],
            "vt": vt_b[b],
            "mt": np.ascontiguousarray(mask[b, rows, :].T.astype(fp8)),
            "wq": wq_t, "wk": wk_t, "wv": wv_t,
            "bq": bqc, "bk": bkc, "bv": bvc,
        })

    nc = _build()
    res = run_bass_kernel_spmd(nc, in_maps, core_ids=list(range(N_CORES)))
    _last_results = res

    out = np.empty((B, S, DK), dtype=np.float32)
    for c in range(N_CORES):
        b, i = divmod(c, GROUP)
        out[b, i * SQ:(i + 1) * SQ, :] = res.results[c]["out"]
    return out
